# revision 30
# baseline (speedup 1.0000x reference)
"""EMA kernel for Trainium2 (Bass/Tile), 8-core SPMD, fp16 IO.

Problem: a[b, c, 0] = x[b, c, 0]
         a[b, c, t] = w[c] * x[b, c, t] + (1 - w[c]) * a[b, c, t-1]
         output[b, t, c] = a[b, c, t],  w = clip(weights, 0, 0.2)

The kernel is DMA-bound (358 GB/s/core cost-model bandwidth), so IO is
shipped as fp16 while the recurrence state stays fp32. Rather than running
the scan on w*x (which costs a full elementwise multiply pass), it scans

    u_t = (1-w) * u_{t-1} + x_t,   u_0 = x_0 / w

so that a_t = w * u_t exactly, and the *w* multiply rides along inside the
PE transpose for free: real 128x128 matmuls against a diag(w) stationary
(out = u_blk.T @ diag(w)) transpose and scale at once. The fp16 round of u
is a relative error, and a = w*u, so the output keeps ~2^-11 relative
accuracy; channels with w < 2e-4 (absent for any sane input; guards seed
overflow) are patched on the host with the closed form a_t ~ (1-w)^t x_0.

Per core (B sharded 8 ways -> 8 batches/core), per batch, per channel-half:
  - x tile fp16 [128 chans (partitions), 2048 t]     (256KB DMA, SP-issued)
  - DVE : tensor_tensor_scan  state = (1-w)*state + x (f32 state, fp16 out;
          host-computed seeds x_0/w ride in with the first packed x load)
  - PE  : fp16 [128,128] matmuls u_blk.T @ diag(w) -> PSUM f32 (the
          dedicated transpose path ignores its stationary, so a real
          matmul does transpose+scale at the same 1 cycle/row)
  - ACT : convert-copy PSUM f32 -> SBUF fp16 staging (diag(w) itself is
          built on-device with affine_select on the otherwise-idle Pool)
  - Pool: SWDGE-issued 256KB stores, 512B contiguous rows of [t, c]
"""

from contextlib import ExitStack

import numpy as np

import concourse.bass as bass
import concourse.tile as tile
from concourse import mybir
from concourse.bass_utils import run_bass_kernel_spmd

B, C, T = 64, 256, 2048
N_CORES = 8
B_LOC = B // N_CORES  # 8 batches per core
P = 128
NH = C // P  # 2 channel halves
NTB = T // P  # 16 time blocks
F32 = mybir.dt.float32
F16 = mybir.dt.float16
W_MIN = 2e-4  # below this, the u-scan seed risks fp16 overflow; host patches


def build_nc():
    nc = bass.Bass()
    x = nc.dram_tensor("x", [B_LOC, C, T], F16, kind="ExternalInput")
    # xc packs the per-core constants INTO the first x load (cols, fp16):
    #   [0:4)   (1-w) halves as f32 bit-pairs (scan decay, must be exact)
    #   [4:6)   w halves fp16 (diag build)
    #   [6:22)  seeds x_0/w fp16, one col per (batch, half)
    #   [22:)   x[b0, half0]
    # One DMA, full-rate 4KB+ rows -- the standalone 80B-row consts DMA paid
    # the 7ns/descriptor floor.
    NCC = 22 + T
    xc = nc.dram_tensor("xc", [P, NCC], F16, kind="ExternalInput")
    out = nc.dram_tensor("out", [B_LOC, T, C], F16, kind="ExternalOutput")

    with tile.TileContext(nc) as tc, ExitStack() as ctx:
        consts = ctx.enter_context(tc.tile_pool(name="consts", bufs=1))
        # xp depth paces loads: load k+8 blocks on tile reuse until the
        # batch consuming tile k is scanned (SP only issues loads, so the
        # stall blocks nothing else)
        xp = ctx.enter_context(tc.tile_pool(name="xp", bufs=8))
        upool = ctx.enter_context(tc.tile_pool(name="upool", bufs=8))
        stage = ctx.enter_context(tc.tile_pool(name="stage", bufs=10))
        psum = ctx.enter_context(tc.tile_pool(name="psum", bufs=4, space="PSUM"))

        ct = consts.tile([P, NCC], F16)
        nc.sync.dma_start(out=ct, in_=xc[:, :])
        wt = ct[:, 0:4].bitcast(F32)  # [P, 2] f32: (1-w) per half
        w16 = ct[:, 4:6]
        seeds = ct[:, 6:22]
        x00 = ct[:, 22:]
        # diag(w) built on-device (Pool is idle; saves the 64KB wdiag DMA):
        # wd_t[p, h, j] = w16[p, h] where j == p, else 0
        wd_t = consts.tile([P, NH, P], F16)
        for h in range(NH):
            nc.gpsimd.affine_select(
                out=wd_t[:, h, :],
                in_=w16[:, h : h + 1].to_broadcast((P, P)),
                pattern=[[1, P]],
                compare_op=mybir.AluOpType.is_equal,
                fill=0.0,
                base=0,
                channel_multiplier=-1,
            )

        # Hoist every x load to the front (SP-issued) so the (exclusive)
        # DMA device services loads early and compute never starves; the
        # xp pool depth paces the last loads into the gaps between stores.
        x_tiles = [x00]  # batch 0 half 0 rides in with the consts
        for b in range(B_LOC):
            xr = x[b].rearrange("(h p) t -> p h t", p=P)
            for h in range(NH):
                if b == 0 and h == 0:
                    continue
                x_t = xp.tile([P, T], F16, tag="x")
                nc.sync.dma_start(out=x_t, in_=xr[:, h, :])
                x_tiles.append(x_t)

        for b in range(B_LOC):
            u_tiles = []
            for h in range(NH):
                x_t = x_tiles[b * NH + h]
                # u_t = (1-w)*u_{t-1} + x_t ; f32 state, fp16 out
                u_t = upool.tile([P, T], F16, tag="u")
                nc.vector.tensor_tensor_scan(
                    out=u_t,
                    data0=wt[:, h : h + 1].to_broadcast((P, T)),
                    data1=x_t,
                    initial=seeds[:, b * NH + h : b * NH + h + 1],
                    op0=mybir.AluOpType.mult,
                    op1=mybir.AluOpType.add,
                )
                u_tiles.append(u_t)

            for tbg in range(2):  # halves of T
                st = stage.tile([P, 8, C], F16)
                for tp in range(2):  # 2-bank f32 PSUM tiles, 4 t-blocks each
                    ps = psum.tile([P, 4, NH, P], F32)
                    for sub in range(4):
                        tb = tbg * 8 + tp * 4 + sub
                        for h in range(NH):
                            # a.T = u_blk.T @ diag(w): transpose + scale in
                            # one real matmul (NOT is_transpose: the HW
                            # transpose path ignores its stationary matrix)
                            nc.tensor.matmul(
                                ps[:, sub, h, :],
                                u_tiles[h][:, tb * P : (tb + 1) * P],
                                wd_t[:, h, :],
                                start=True,
                                stop=True,
                            )
                    # ACT converts f32 PSUM -> fp16 stage (only ACT/DVE
                    # may read PSUM); DVE, idle after its final scan, takes
                    # half the last batch's copies to shorten the tail
                    dstv = st[:, tp * 4 : (tp + 1) * 4, :].rearrange(
                        "p a b -> p (a b)"
                    )
                    srcv = ps.rearrange("p a b c -> p (a b c)")
                    if b == B_LOC - 1 and tp == 1:
                        nc.vector.tensor_copy(out=dstv, in_=srcv)
                    else:
                        nc.scalar.copy(out=dstv, in_=srcv)
                # 256KB store per half-T: 8 t-blocks x 256 chans, 512B rows.
                # Pool's SWDGE issues stores so SP/ACT sequencers never park
                # on a store's semaphore wait in front of load issues.
                dst = out[
                    b, tbg * 1024 : (tbg + 1) * 1024, :
                ].rearrange("(tb p) c -> p tb c", p=P)
                nc.gpsimd.dma_start(out=dst, in_=st)

    sort_waits_by_resolution(nc)
    move_exit_waits_to_pool(nc)
    split_excess_waits(nc)
    hoist_first_dmas(nc)
    trim_exit_barrier(nc)
    return nc


def move_exit_waits_to_pool(nc):
    """The epilogue's DMA-completion waits sit on SP's drain, in front of a
    multi-hop SP->gather->Pool chain that runs after the final store's
    semaphore lands. Moving the waits onto Pool's last drain (right before
    the end-marker ISA) lets the gather complete early and leaves only
    drain+ISA after the last semaphore."""
    blk = nc.m.functions[0].blocks[-1]
    src = next(
        (
            i
            for i in blk.instructions
            if isinstance(i, mybir.InstDrain)
            and i.engine == mybir.EngineType.SP
            and i.sync_info
            and len(i.sync_info.on_wait) > 1
        ),
        None,
    )
    dst = next(
        (
            i
            for i in reversed(blk.instructions)
            if isinstance(i, mybir.InstDrain)
            and i.engine == mybir.EngineType.Pool
        ),
        None,
    )
    if src is None or dst is None:
        return
    dst_waits = list(dst.sync_info.on_wait) if dst.sync_info else []
    dst.sync_info = mybir.SyncInfo(
        on_wait=dst_waits + list(src.sync_info.on_wait),
        on_update=dst.sync_info.on_update if dst.sync_info else [],
    )
    src.sync_info = mybir.SyncInfo(
        on_wait=[], on_update=src.sync_info.on_update
    )


def sort_waits_by_resolution(nc):
    """Order each instruction's semaphore waits by the program order of the
    waited semaphore's last updater. split_excess_waits() then emits the
    early-resolving waits as standalone EventSemaphores that drain while the
    late ones are still pending, so only the truly binding wait (kept inline)
    sits after the last semaphore resolves -- instead of a 50ns-per-wait
    decode chain following it."""
    last_upd = {}
    order = 0
    for f in nc.m.functions:
        for blk in f.blocks:
            for ins in blk.instructions:
                si = ins.sync_info
                if si and si.on_update:
                    for u in si.on_update:
                        last_upd[u.id] = order
                order += 1
    for f in nc.m.functions:
        for blk in f.blocks:
            for ins in blk.instructions:
                si = ins.sync_info
                if si and si.on_wait and len(si.on_wait) > 1:
                    waits = sorted(
                        si.on_wait, key=lambda w: last_upd.get(w.id, -1)
                    )
                    ins.sync_info = mybir.SyncInfo(
                        on_wait=waits, on_update=si.on_update
                    )


def trim_exit_barrier(nc):
    """Drop the second Drain/EventSemaphore barrier round at the end of the
    epilogue block. The first round already waits on every outstanding DMA
    semaphore (including the final store), so the second gather is pure
    ceremony on the measured timeline."""
    blk = nc.m.functions[0].blocks[-1]
    isa_idx = max(
        (k for k, i in enumerate(blk.instructions)
         if isinstance(i, mybir.InstISA)),
        default=None,
    )
    if isa_idx is None:
        return
    tail = blk.instructions[isa_idx + 1 :]
    if all(
        isinstance(i, (mybir.InstDrain, mybir.InstEventSemaphore))
        for i in tail
    ):
        del blk.instructions[isa_idx + 1 :]


def hoist_first_dmas(nc):
    """Move the first SP x-load and the ACT wseed load ahead of the entry
    barrier (engine Drain + gather EventSemaphore) in the preamble block.
    Both have no semaphore waits, and their completion increments fire long
    after every engine has passed the barrier, so ordering is preserved while
    the first DMA transfer starts ~700ns earlier."""
    fn = nc.m.functions[0]
    if len(fn.blocks) < 2:
        return
    pre, main = fn.blocks[0], fn.blocks[1]
    for eng in (mybir.EngineType.SP, mybir.EngineType.Activation):
        dma = next(
            (
                i
                for i in main.instructions
                if isinstance(i, mybir.InstDMACopy) and i.engine == eng
            ),
            None,
        )
        if dma is None or (dma.sync_info and dma.sync_info.on_wait):
            continue
        drain_idx = next(
            (
                k
                for k, i in enumerate(pre.instructions)
                if isinstance(i, mybir.InstDrain) and i.engine == eng
            ),
            None,
        )
        if drain_idx is None:
            continue
        main.instructions.remove(dma)
        if eng == mybir.EngineType.SP:
            # SP's preamble regmoves only set the zero reg and disable the
            # bounds-check regs; the load references no registers, so it can
            # issue first of all
            first_sp = next(
                k
                for k, i in enumerate(pre.instructions)
                if i.engine == eng
            )
            pre.instructions.insert(first_sp, dma)
        else:
            pre.instructions.insert(drain_idx, dma)


def split_excess_waits(nc, cap=1):
    """Hoist all but `cap` semaphore waits of each instruction into standalone
    EventSemaphore instructions right before it (walrus's setupSyncWait only
    encodes one wait per TPB instruction)."""
    n_split = 0
    for f in nc.m.functions:
        for blk in f.blocks:
            new_insts = []
            for ins in blk.instructions:
                si = ins.sync_info
                waits = list(si.on_wait) if si and si.on_wait else []
                if len(waits) > cap:
                    for i, w in enumerate(waits[:-cap]):
                        es = mybir.InstEventSemaphore(
                            name=f"{ins.name}-esw{i}", ins=[], outs=[]
                        )
                        es.engine = ins.engine
                        es.sync_info = mybir.SyncInfo(on_wait=[w], on_update=[])
                        new_insts.append(es)
                        n_split += 1
                    ins.sync_info = mybir.SyncInfo(
                        on_wait=waits[-cap:], on_update=si.on_update
                    )
                new_insts.append(ins)
            blk.instructions = new_insts
    return n_split


_NC_CACHE = []


def _get_nc():
    if not _NC_CACHE:
        _NC_CACHE.append(build_nc())
    return _NC_CACHE[0]


def _prep_weights(weights):
    w = np.clip(np.asarray(weights, dtype=np.float64), 0.0, 0.2)
    bad = w < W_MIN  # u-scan seed would overflow; host patches these channels
    winv = np.where(bad, 0.0, 1.0 / np.maximum(w, W_MIN))
    onemw = 1.0 - w
    onemw32 = np.stack([onemw[:P], onemw[P:]], axis=1).astype(np.float32)
    # f32 bit-pairs viewed as fp16 cols [P, 4], then w as fp16 [P, 2]
    wtab = np.concatenate(
        [
            onemw32.view(np.float16),
            np.stack([w[:P], w[P:]], axis=1).astype(np.float16),
        ],
        axis=1,
    )  # [P, 6] fp16
    return w, bad, np.ascontiguousarray(wtab)


def _make_in_maps(x16, wtab, seeds):
    maps = []
    for i in range(N_CORES):
        xi = x16[i * B_LOC : (i + 1) * B_LOC]
        xc = np.concatenate(
            [wtab, seeds[i], xi[0, :P, :]], axis=1
        )  # [P, 22 + T] fp16
        maps.append(
            {
                "x": np.ascontiguousarray(xi),
                "xc": np.ascontiguousarray(xc),
            }
        )
    return maps


def run(x, weights, **run_kwargs):
    nc = _get_nc()
    x16 = np.asarray(x, dtype=np.float32).astype(np.float16)
    w, bad, wtab = _prep_weights(weights)
    winv = np.where(bad, 0.0, 1.0 / np.maximum(w, W_MIN))  # [C] f64
    # seeds[core][p, b*NH+h] = x[global_b, h*128+p, 0] * winv, from f32 x
    x0 = np.asarray(x, dtype=np.float64)[:, :, 0] * winv[None, :]  # [B, C]
    seeds = np.ascontiguousarray(
        x0.reshape(N_CORES, B_LOC, NH, P).transpose(0, 3, 1, 2).reshape(
            N_CORES, P, B_LOC * NH
        ).astype(np.float16)
    )
    res = run_bass_kernel_spmd(
        nc, _make_in_maps(x16, wtab, seeds), core_ids=list(range(N_CORES)),
        **run_kwargs,
    )
    full = np.concatenate([r["out"] for r in res.results], axis=0).astype(
        np.float32
    )  # [B, T, C]
    if bad.any():
        # w ~ 0: a_t = (1-w)^t x_0 + O(w); the fluctuation term is below the
        # fp16 noise floor, so the closed form is the better answer.
        idx = np.nonzero(bad)[0]
        x0 = np.asarray(x, dtype=np.float64)[:, idx, 0]  # [B, nbad]
        decay = (1.0 - w[idx])[None, :] ** np.arange(T)[:, None]  # [T, nbad]
        full[:, :, idx] = (x0[:, None, :] * decay[None, :, :]).astype(
            np.float32
        )
    return full, res


def kernel(x, initial_state=None, weights=None):
    # initial_state is accepted but unused (matches the reference module).
    full, _ = run(x, weights)
    return full
